# revision 72
# baseline (speedup 1.0000x reference)
"""Trainium2 Bass kernel for nn_CIFAR10Net LIF conv layer.

Reference computation:
  w' = weight-standardized clip(weight) ; conv2d(x, w', pad=1) over (T*B) frames
  LIF scan over T with state (u, sg) [sm/ss are dead state]:
     sg = (sg + I) * (1 - 1/tau_grad);  u = u + sg
     spike = u >= th ; u, sg *= (1 - spike)
Spikes out: [T, B, 128, 32, 32] f32.

Device mapping (per core, B sharded 4/core over 8 cores; each timestep's
4096 positions run as four pipelined psum "quarters" of [128 x 1024]):
  - PE conv in fp16 split precision, 2 cyc/row total: per batch, term Wh@Xh
    (27-row) plus ONE merged matmul [Wl;Wh]@[Xh;Xl] (54-row stacked
    contraction = both cross terms; Wl@Xl ~2^-22 dropped). Batches pair
    two-per-im-plane at 64-row tile positions (fmap/weight same-start rule).
    The recurrence adds Id @ h_{t-1} (exact identity, f32r) so psum = sg'_t.
  - ACT: gs = cg * sg' copied PSUM->SBUF (scale folds cg: keeps the identity
    exact and every DVE op SBUF-only, releasing psum tiles early).
  - DVE custom LIF_U2: u''_t = select(cg*u + gs < cg*th, u + gs/cg, 0); the
    threshold is a float immediate when uniform ([P,1]-AP fallback otherwise).
  - quarters 0-2: DVE mask m = (u'' != 0) int8 {0,1} (spike output AND Pool
    operand); Pool tensor_tensor mult: h''_t = m * gs = cg * masked sg'.
    quarter 3: fused DVE LIF_SG2 h'' = select(.., gs, 0); spike via ACT Sign,
    deferred one timestep so it stays off the ACT-copy chain.
  - spikes decode host-side as (s == 0) for both encodings.
  - startup: PE-warmup matmuls cover the clock ramp, the ACT table preloads
    from warmup scratch, and the t=0 input DMA is split by fp16 plane.
"""

import os
import numpy as np

import concourse.bacc as bacc
import concourse.mybir as mybir
import concourse.dve_ops as dve_ops
from concourse.dve_spec import Spec, Src0, Src1, C0, C1, C2, Zero, select, lower
from concourse.dve_uop import DveOpSpec
from concourse.tile import TileContext
from concourse.bass_utils import run_bass_kernel_spmd

# ---------------- constants -------------------------------------------------
T, B, CIN, H, W = 16, 32, 3, 32, 32
COUT, KK = 128, 3
NCORES = 8
BSH = B // NCORES          # 4 batches per core
CG = np.float32(1.0 - 1.0 / 3.5)
NB = 512                   # positions per psum bank (= one batch half)
NHALF = 4 * NB             # 2048 positions per half-step
NQ = 2 * NB                # 1024 positions per psum quarter-tile
CONV_MODE = os.environ.get("LIF_CONV_MODE", "fp16x3")  # fp16x3 | f32
IDDT_NAME = os.environ.get("LIF_ID_DTYPE", "f32r")     # f32 | f32r
ODMA_ENG = os.environ.get("LIF_ODMA_ENG", "sync")    # gpsimd|sync|scalar|vector
KREPEAT = int(os.environ.get("LIF_KREPEAT", "1"))
ABLATE = set(filter(None, os.environ.get("LIF_ABLATE", "").split(",")))

# ---------------- custom DVE ops -------------------------------------------
_s = Src0 + Src1


def _register_op(name, spec):
    shas = {}
    for ver in ("v3",):
        uops = lower(spec, ver=ver)
        shas[ver] = DveOpSpec(name=name, opcode=0, uops=uops, rd1_en=True).sha(ver)
    op = dve_ops.DveOp(name, spec, subdim=False, uops_sha=shas)
    for o in dve_ops.OPS:
        if o.name == name:
            return o
    dve_ops.OPS.append(op)
    dve_ops.CUSTOM_DVE_SPECS[name] = spec
    dve_ops._SUB_OPCODE_FOR_NAME[name] = max(dve_ops._SUB_OPCODE_FOR_NAME.values()) + 1
    assert dve_ops._SUB_OPCODE_FOR_NAME[name] < 0x20
    return op


# gs-based ops: in1 = gs = cg*gamma (SBUF), s0 = cg*th, s1 = cg, imm2 = 1/cg.
# Compare is the exact rescale  u + gamma < th  <=>  cg*u + gs < cg*th.
_sc = Src0 * C1 + Src1
LIF_SG = _register_op(
    "LIF_SG2",
    Spec(
        body=select(_sc < C0, Src1, Zero),
        reference=lambda in0, in1, s0, s1, imm2: np.where(
            (in0 * s1 + in1) < s0, in1, 0.0
        ).astype(np.float32),
    ),
)
LIF_U = _register_op(
    "LIF_U2",
    Spec(
        body=select(_sc < C0, Src0 + Src1 * C2, Zero),
        reference=lambda in0, in1, s0, s1, imm2: np.where(
            (in0 * s1 + in1) < s0, (in0 + in1 * imm2).astype(np.float32), 0.0
        ).astype(np.float32),
    ),
)

# ---------------- device kernel builder -------------------------------------
_NC_CACHE = {}


def _build_nc(krepeat=None, th_imm=None):
    krepeat = KREPEAT if krepeat is None else krepeat
    key = (CONV_MODE, krepeat, IDDT_NAME, ODMA_ENG, th_imm, tuple(sorted(ABLATE)))
    if key in _NC_CACHE:
        return _NC_CACHE[key]
    f32 = mybir.dt.float32
    f16 = mybir.dt.float16
    iddt = mybir.dt.float32r if IDDT_NAME == "f32r" else f32
    fp16conv = CONV_MODE == "fp16x3"
    nc = bacc.Bacc("TRN2", target_bir_lowering=False)

    if fp16conv:
        # im2col planes: plane p holds batches {2p, 2p+1} in 64-row groups,
        # rows 64g+r: r<27 Xh, 27<=r<54 Xl of batch 2p+g.
        xpad = nc.dram_tensor("xpad", [T, COUT, 2, 1156], f16, kind="ExternalInput")
        wmat = nc.dram_tensor("wmat", [COUT, 2 * COUT], f16, kind="ExternalInput")
    else:
        xpad = nc.dram_tensor("xpad", [T, COUT, 1156], f32, kind="ExternalInput")
        wmat = nc.dram_tensor("wmat", [COUT, COUT], f32, kind="ExternalInput")
    cgid = nc.dram_tensor("cgid", [COUT, COUT], iddt, kind="ExternalInput")
    if th_imm is None:
        th = nc.dram_tensor("th", [COUT, 1], f32, kind="ExternalInput")
    spk = nc.dram_tensor(
        "spk", [T, 2, COUT, NHALF], mybir.dt.int8, kind="ExternalOutput"
    )

    with TileContext(nc) as tc, \
         tc.tile_pool(name="const", bufs=1) as cpool, \
         tc.tile_pool(name="state", bufs=1) as spool, \
         tc.tile_pool(name="im", bufs=6) as impool, \
         tc.tile_pool(name="out", bufs=6) as opool, \
         tc.tile_pool(name="gs", bufs=6) as gspool, \
         tc.tile_pool(name="ps", bufs=4, space="PSUM") as ppool:

        if fp16conv:
            # cols 0:128: [Wl;Wh] merged stationary at rows {0,64}+0:54;
            # cols 128:256: Wh at rows {0,64}+0:27 (fmap/weight same-start rule)
            w2_sb = cpool.tile([COUT, 2 * COUT], f16, tag="w2")
        else:
            w_sb = cpool.tile([COUT, COUT], f32, tag="w")
            nc.sync.dma_start(w_sb[:], wmat[:])
        id_sb = cpool.tile([COUT, COUT], iddt, tag="id")
        th_sb = cpool.tile([COUT, 1], f32, tag="th")
        if fp16conv:
            nc.scalar.dma_start(w2_sb[:], wmat[:])
        else:
            nc.scalar.dma_start(w_sb[:], wmat[:])
        nc.scalar.dma_start(th_sb[:], th[:])
        nc.scalar.dma_start(id_sb[:], cgid[:])

        # PE clock warmup: dummy fp16 matmuls with no DMA dependency keep the
        # PE busy through its ramp window while the first inputs stream in.
        wu = cpool.tile([COUT, NB], f16, tag="wu")
        nc.gpsimd.memset(wu[:], 0.0)
        ps_w = ppool.tile([COUT, NQ], f32, tag="ps", name="ps_w")
        for i in range(9):
            nc.tensor.matmul(
                ps_w[:, 0:NB], wu[:, 0:COUT], wu[:],
                start=True, stop=True, skip_group_check=True,
            )

        odma = {"gpsimd": nc.gpsimd, "sync": nc.sync, "scalar": nc.scalar,
                "vector": nc.vector}[ODMA_ENG]

        ubuf = [spool.tile([COUT, 2 * NHALF], f32, tag=f"u{i}", name=f"u{i}") for i in range(2)]
        gbuf = [spool.tile([COUT, 2 * NHALF], iddt, tag=f"g{i}", name=f"g{i}") for i in range(2)]
        for _rep in range(krepeat):
          nc.gpsimd.memset(ubuf[0][:], 0.0)

          pending_sign = None
          for t in range(T):
              ucur, unext = ubuf[t % 2], ubuf[(t + 1) % 2]
              gcur, gnext = gbuf[t % 2], gbuf[(t + 1) % 2]

              if pending_sign is not None:
                  # q3 spike of t-1: input long ready, so it runs with no wait
                  # and stays out of the ACT-copy chain of this timestep
                  _st, _src_ap, _dst_ap = pending_sign
                  nc.scalar.activation(
                      _st[:], _src_ap, mybir.ActivationFunctionType.Sign
                  )
                  if "outdma" not in ABLATE:
                      odma.dma_start(_dst_ap, _st[:])
                  pending_sign = None

              if fp16conv:
                  im = impool.tile([COUT, 2, 34, 34], f16, tag="im27")
              else:
                  im = impool.tile([COUT, 34, 34], f32, tag="im27")
              if "imdma" in ABLATE:
                  nc.vector.memset(im[:], 0.0)
              else:
                  eng = nc.sync if t % 2 == 0 else nc.scalar
                  if fp16conv and t == 0:
                      # plane 0 (quarters bp=0) lands first at startup
                      eng.dma_start(im[:, 0], xpad[t, :, 0])
                      eng.dma_start(im[:, 1], xpad[t, :, 1])
                  else:
                      eng.dma_start(im[:], xpad[t, :])

              for q in range(4):
                  half, bp = q // 2, q % 2
                  lo = half * NHALF + bp * NQ
                  ps = ppool.tile([COUT, NQ], f32, tag="ps")
                  for j in range(2) if "conv" not in ABLATE else []:
                      b = 2 * bp + j
                      if fp16conv:
                          # batch b lives in plane bp at 64-row group 64*j
                          nc.tensor.matmul(
                              ps[:, NB * j : NB * (j + 1)],
                              w2_sb[64 * j : 64 * j + 27, COUT : 2 * COUT],
                              im[64 * j : 64 * j + 27, bp, 16 * half : 16 * half + 16, 0:32],
                              start=True,
                              stop=False,
                              tile_position=(64 * j, 0),
                              skip_group_check=True,
                          )
                          nc.tensor.matmul(
                              ps[:, NB * j : NB * (j + 1)],
                              w2_sb[64 * j : 64 * j + 54, 0:COUT],
                              im[64 * j : 64 * j + 54, bp, 16 * half : 16 * half + 16, 0:32],
                              start=False,
                              stop=(t == 0),
                              tile_position=(64 * j, 0),
                              skip_group_check=True,
                          )
                      else:
                          nc.tensor.matmul(
                              ps[:, NB * j : NB * (j + 1)],
                              w_sb[32 * b : 32 * b + 27, :],
                              im[32 * b : 32 * b + 27, 16 * half : 16 * half + 16, 0:32],
                              start=True,
                              stop=(t == 0),
                              tile_position=(32 * b, 0),
                              skip_group_check=True,
                          )
                  for j in (range(2) if ("idmm" not in ABLATE and t > 0) else []):
                      nc.tensor.matmul(
                          ps[:, NB * j : NB * (j + 1)],
                          id_sb[:],
                          gcur[:, lo + NB * j : lo + NB * (j + 1)],
                          start=False,
                          stop=True,
                          tile_position=(0, 0),
                          skip_group_check=True,
                      )

                  # cg*sg' copy PSUM -> SBUF on ACT (Pool cannot read
                  # PSUM); cg fold keeps the idmm weights an exact identity
                  # and lets every DVE op run SBUF-only (psum freed early)
                  gs = gspool.tile([COUT, NQ], f32, tag="gs")
                  nc.scalar.activation(
                      gs[:], ps[:], mybir.ActivationFunctionType.Identity,
                      scale=float(CG),
                  )
                  if "dve" in ABLATE:
                      nc.vector.memset(unext[:, lo : lo + NQ], 0.0)
                  else:
                      nc.vector._custom_dve(
                          LIF_U,
                          out=unext[:, lo : lo + NQ],
                          in0=ucur[:, lo : lo + NQ],
                          in1=gs[:],
                          s0=(s0_arg[:] if th_imm is None else s0_arg),
                          s1=float(CG),
                          imm2=float(1.0 / CG),
                      )
                  st = opool.tile([COUT, NQ], mybir.dt.int8, tag="spk")
                  if q < 3:
                      # survive mask m = (u'' != 0) int8 {0,1}: doubles as the
                      # spike output (spike <=> s==0) and Pool's multiplicand
                      nc.vector.tensor_scalar(
                          st[:], unext[:, lo : lo + NQ], 0.0, None,
                          mybir.AluOpType.not_equal,
                      )
                      # h''_t = m * cg*sg'  (Pool tensor_tensor, all-SBUF);
                      # dead at the last step, nothing consumes it
                      if "gsel" in ABLATE:
                          nc.gpsimd.memset(gnext[:, lo : lo + NQ], 0.0)
                      elif t < T - 1:
                          nc.gpsimd.tensor_tensor(
                              gnext[:, lo : lo + NQ],
                              st[:],
                              gs[:],
                              mybir.AluOpType.mult,
                          )
                  else:
                      # fused path: h''_t = select(u+sg' < th, cg*sg', 0) on
                      # DVE; dead at the last step. Spike via ACT Sign (s==0)
                      if t < T - 1:
                          nc.vector._custom_dve(
                              LIF_SG,
                              out=gnext[:, lo : lo + NQ],
                              in0=ucur[:, lo : lo + NQ],
                              in1=gs[:],
                              s0=(s0_arg[:] if th_imm is None else s0_arg),
                              s1=float(CG),
                          )
                      pending_sign = (
                          st,
                          unext[:, lo : lo + NQ],
                          spk[t, half, :, bp * NQ : (bp + 1) * NQ],
                      )
                  if "outdma" not in ABLATE and q < 3:
                      odma.dma_start(spk[t, half, :, bp * NQ : (bp + 1) * NQ], st[:])

          if pending_sign is not None:
              _st, _src_ap, _dst_ap = pending_sign
              nc.scalar.activation(
                  _st[:], _src_ap, mybir.ActivationFunctionType.Sign
              )
              if "outdma" not in ABLATE:
                  odma.dma_start(_dst_ap, _st[:])
              pending_sign = None

    nc.finalize()
    _NC_CACHE[key] = nc
    return nc


# ---------------- host side --------------------------------------------------
def _prep_weights(weight, norm_weight, norm_bias):
    w = np.clip(weight.astype(np.float32), -4.0, 4.0)
    flat = w.reshape(COUT, -1)
    mean = flat.mean(axis=1, dtype=np.float32)
    var = flat.var(axis=1, ddof=1, dtype=np.float32)
    scale = (norm_weight.reshape(COUT).astype(np.float32)
             / np.sqrt(var + np.float32(1e-5)))
    w_std = (w - mean[:, None, None, None]) * scale[:, None, None, None] \
        + norm_bias.reshape(COUT, 1, 1, 1).astype(np.float32)
    # wk27[3*(3dy+dx) + c, co] = cg * w_std[co, c, dy, dx]
    wk27 = np.zeros((27, COUT), np.float32)
    wk = (CG * w_std).transpose(1, 2, 3, 0)  # [c, dy, dx, co]
    for dy in range(3):
        for dx in range(3):
            r = 3 * (3 * dy + dx)
            wk27[r : r + 3, :] = wk[:, dy, dx, :]
    return wk27


def _im2col(x):
    xp = np.pad(x, [(0, 0), (0, 0), (0, 0), (1, 1), (1, 1)])  # [T,B,C,34,34]
    # x27[t, 32b + 3*(3dy+dx) + c, f] = xpad[t, b, c].flat[34*dy + dx + f]
    xflat = np.pad(xp.reshape(T, B, CIN * 1156), [(0, 0), (0, 0), (0, 128)])
    x27 = np.zeros((T, B, 32, 1156), np.float32)
    for dy in range(3):
        for dx in range(3):
            for c in range(CIN):
                off = c * 1156 + 34 * dy + dx
                x27[:, :, 3 * (3 * dy + dx) + c, :] = xflat[:, :, off : off + 1156]
    return x27


def kernel(x, weight, norm_weight, norm_bias, threshold, _want_trace=False, _krepeat=None):
    x = np.asarray(x, np.float32)
    th_r = np.asarray(threshold, np.float32).reshape(COUT)
    th_imm = float(th_r[0]) if np.all(th_r == th_r[0]) else None
    nc = _build_nc(_krepeat, th_imm=th_imm)
    wmat = _prep_weights(np.asarray(weight), np.asarray(norm_weight),
                         np.asarray(norm_bias))
    cgid = np.eye(COUT, dtype=np.float32)
    th_h = (CG * th_r.reshape(COUT, 1)).astype(np.float32)

    x27 = _im2col(x)
    fp16conv = CONV_MODE == "fp16x3"
    if fp16conv:
        wh = wmat.astype(np.float16)          # [27, 128]
        wl = (wmat - wh.astype(np.float32)).astype(np.float16)
        w_in = np.zeros((COUT, 2 * COUT), np.float16)
        for g in (0, 64):                     # same-start copies per 64-group
            w_in[g : g + 27, 0:COUT] = wl     # merged term: [Wl; Wh]
            w_in[g + 27 : g + 54, 0:COUT] = wh
            w_in[g : g + 27, COUT:] = wh      # plain term stationary
    else:
        w_in = np.zeros((COUT, COUT), np.float32)
        for b in range(BSH):
            w_in[32 * b : 32 * b + 27] = wmat

    in_maps = []
    for core in range(NCORES):
        xs = x27[:, core * BSH : (core + 1) * BSH]  # [T, 4, 32, 1156]
        if fp16conv:
            xh = xs.astype(np.float16)
            xl = (xs - xh.astype(np.float32)).astype(np.float16)
            # plane p holds batches {2p, 2p+1} in 64-row groups [Xh(27); Xl(27)]
            xi = np.zeros((T, COUT, 2, 1156), np.float16)
            for p in range(2):
                for g in range(2):
                    b = 2 * p + g
                    xi[:, 64 * g : 64 * g + 27, p] = xh[:, b, 0:27]
                    xi[:, 64 * g + 27 : 64 * g + 54, p] = xl[:, b, 0:27]
            xs = np.ascontiguousarray(xi)
        else:
            xs = np.ascontiguousarray(xs.reshape(T, COUT, 1156))
        m = {"xpad": xs, "wmat": w_in, "cgid": cgid}
        if th_imm is None:
            m["th"] = th_h
        in_maps.append(m)

    res = run_bass_kernel_spmd(
        nc, in_maps, core_ids=list(range(NCORES)), trace=_want_trace
    )

    out = np.empty((T, B, COUT, H, W), np.float32)
    for core in range(NCORES):
        s = res.results[core]["spk"]  # [T, 2, 128, 2048]
        spikes = (s == 0)
        # [t, half, co, b, hh, w] -> [t, b, co, 16*half+hh, w]
        spikes = spikes.reshape(T, 2, COUT, BSH, 16, W).transpose(0, 3, 2, 1, 4, 5)
        out[:, core * BSH : (core + 1) * BSH] = spikes.reshape(
            T, BSH, COUT, H, W
        ).astype(np.float32)
    if _want_trace:
        kernel.last_result = res
    return out
